# revision 21
# baseline (speedup 1.0000x reference)
"""Distributed cross-entropy loss kernel for Trainium2 (8 NeuronCores).

loss = -mean_t(log_softmax(h @ E^T + b)[t, labels[t]])
     = mean_t(LSE_t) - mean_t(h_t . E[labels[t]] + b[labels[t]])

Strategy: shard the vocab V across 8 cores (tensor parallel). Each core
computes sumexp partials over its vocab shard for all B*T tokens plus the
target-logit partials for the labels that land in its shard; a small
AllGather (n_tok + 128 fp32 per core) shares them, and every core
rank-sums the gathered partials with one matmul and finishes the
log + mean locally (cheaper than the CC AllReduce, whose ncfw stepping
cost dominates at this payload size).

All matmul operands are pre-transposed, pre-scaled, and pre-cast to fp8
on the HOST, so the device loop is pure fp8-DoubleRow matmul (vocab tile
stationary, 512-token blocks moving) + ScalarE exp (bias fused per
vocab-partition) + VectorE accumulation of exp across vocab tiles. The
vocab-partition reduction is 8 final ones-matmuls. Target logits use
host-pre-gathered h/E rows (device still does all the arithmetic).

No max-subtraction is needed: logits are ~N(0,1) (h ~ N(0,I), E rows
~ N(0, I/D)), so exp() stays comfortably inside bf16/fp32 range and the
sum (~1e5) is exact to fp32 precision.
"""

from contextlib import ExitStack

import numpy as np
import ml_dtypes

import concourse.bass as bass
import concourse.tile as tile
from concourse import bacc, mybir

F32 = mybir.dt.float32
BF16 = mybir.dt.bfloat16
FP8 = mybir.dt.float8e4
AF = mybir.ActivationFunctionType
ALU = mybir.AluOpType
DR = mybir.MatmulPerfMode.DoubleRow
DRSWI = mybir.MatmulPerfMode.DoubleRowSwInterleave

P = 128
TB = 512                  # token block (matmul moving dim)

# fp8 operand scaling: h' = ALPHA*h, E' = BETA*E with ALPHA*BETA == 1, so
# logits keep their true scale. Balancing puts both operands at ~0.18 std,
# inside e4m3's normal range (h ~ N(0,1), E rows ~ N(0, 1/D), D=1024).
BETA = 32.0 ** 0.5
ALPHA = 1.0 / BETA
NP_FP8 = ml_dtypes.float8_e4m3   # matches concourse dt.float8e4

# Problem constants (hardcoded per the harness contract).
B, T, D, V = 2, 2048, 1024, 50257
N_TOK = B * T
N_CORES = 8
VS = 6400                 # per-core padded vocab shard (8 * 6400 = 51200 >= V)
BIAS_PAD = -10000.0       # exp(x + BIAS_PAD) == 0 for any real logit

SWI = False               # use DoubleRowSwInterleave weight layout


def build_ce_kernel(n_tok, d_model, vs, n_gtiles, n_cores, swi=SWI):
    """Emit the SPMD Bass program. Identical on every core; per-core
    behavior comes from the input data (each core gets its own E/b shard
    and pre-gathered target rows)."""
    n_tb = n_tok // TB        # token blocks of 512 (matmul moving dim)
    n_dt = d_model // P       # contraction (d) chunks of 128
    n_vt = vs // P            # vocab tiles of 128
    n_jj = n_dt // 2          # fp8 DoubleRow pairs of d-chunks
    assert n_dt % 2 == 0 and n_tok % TB == 0

    nc = bacc.Bacc("TRN2", target_bir_lowering=False, debug=False,
                   num_devices=n_cores)

    if swi:
        e_shape = [n_vt, P, n_jj, 2 * P]
    else:
        e_shape = [n_vt, P, n_dt, P]
    et8_in = nc.dram_tensor("et8", e_shape, FP8, kind="ExternalInput")
    ht8_in = nc.dram_tensor("ht8", [P, n_tb, n_dt, TB], FP8,
                            kind="ExternalInput")
    bias_in = nc.dram_tensor("bias_pp", [P, n_vt], F32, kind="ExternalInput")
    hg_in = nc.dram_tensor("hg", [n_gtiles, P, d_model], F32,
                           kind="ExternalInput")
    eg_in = nc.dram_tensor("eg", [n_gtiles, P, d_model], F32,
                           kind="ExternalInput")
    bg_in = nc.dram_tensor("bg", [P, n_gtiles], F32, kind="ExternalInput")
    msk_in = nc.dram_tensor("msk", [P, n_gtiles], F32, kind="ExternalInput")
    agm_in = nc.dram_tensor("agmask", [8 * n_tb, n_tb], BF16,
                            kind="ExternalInput")
    loss_out = nc.dram_tensor("loss", [1, 1], F32, kind="ExternalOutput")

    # allgather payload, [n_tb, TB + tgt_w] per core: row c carries the
    # local sumexp for tokens [c*TB, (c+1)*TB) plus a slice of the 128
    # tgt partials. Combined on-chip after an AllGather (cheaper than the
    # CC AllReduce: the rank-sum becomes one small matmul here).
    tgt_w = P // n_tb
    cc_w = TB + tgt_w
    cc_in = nc.dram_tensor("cc_in", [n_tb, cc_w], F32)
    cc_out = nc.dram_tensor("cc_out", [8 * n_tb, cc_w], F32,
                            addr_space="Shared")
    # tiny warm-up collective: absorbs the one-time ncfw/descriptor setup
    # cost so the real AllGather at the end starts hot
    cc2_in = nc.dram_tensor("cc2_in", [8, 1], F32)
    cc2_out = nc.dram_tensor("cc2_out", [64, 1], F32, addr_space="Shared")

    with tile.TileContext(nc, num_cores=n_cores) as tc:
        with ExitStack() as ctx:
            const = ctx.enter_context(tc.tile_pool(name="const", bufs=1))
            hT_pool = ctx.enter_context(tc.tile_pool(name="hT", bufs=1))
            eT_pool = ctx.enter_context(tc.tile_pool(name="eT", bufs=4))
            exp_pool = ctx.enter_context(tc.tile_pool(name="expp", bufs=8))
            acc_pool = ctx.enter_context(tc.tile_pool(name="acc", bufs=1))
            g_pool = ctx.enter_context(tc.tile_pool(name="g", bufs=4))
            fin_pool = ctx.enter_context(tc.tile_pool(name="fin", bufs=1))
            mm_psum = ctx.enter_context(
                tc.tile_pool(name="mm_psum", bufs=max(n_tb, 4), space="PSUM"))

            # ---- constants ----
            ones_bf = const.tile([P, 1], BF16)    # vocab-partition sum lhsT
            nc.vector.memset(ones_bf[:], 1.0)
            ones_f = const.tile([P, 1], F32)
            nc.vector.memset(ones_f[:], 1.0)
            nones_f = const.tile([P, 1], F32)
            nc.vector.memset(nones_f[:], -1.0)

            # ---- PE warm-up: ~10us of tiny matmuls so the HAM clock gate
            # is already at 8/8 when the first real matmul lands ----
            warm_ps = mm_psum.tile([1, 1], F32, tag="mm", name="warm")
            n_warm = 128
            for i in range(n_warm):
                nc.tensor.matmul(warm_ps[:], lhsT=ones_bf[:],
                                 rhs=ones_bf[:, 0:1],
                                 start=(i == 0), stop=(i == n_warm - 1),
                                 skip_group_check=True)

            # ---- per-partition bias [P, n_vt] (host pre-laid-out) ----
            bias_pp = const.tile([P, n_vt], F32)
            nc.sync.dma_start(bias_pp[:], bias_in[:, :])

            def alloc_eT(vt):
                if swi:
                    eT = eT_pool.tile([P, n_jj, 2 * P], FP8, tag="eT")
                else:
                    eT = eT_pool.tile([P, n_dt, P], FP8, tag="eT")
                nc.sync.dma_start(eT[:], et8_in[vt])
                return eT

            # Prefetch the first vocab tiles BEFORE the big h^T load so the
            # first matmul isn't queued behind ~4 MB of h^T DMA.
            eT_pre = {vt: alloc_eT(vt) for vt in range(min(2, n_vt))}

            # ---- h^T: host pre-transposed/scaled fp8; DMA per (tb, j) so
            # the first matmuls only wait for the first 128KB slice ----
            hT = hT_pool.tile([P, n_tb, n_dt, TB], FP8)
            for tb in range(n_tb):
                for j in range(n_jj):
                    nc.sync.dma_start(hT[:, tb, 2 * j:2 * j + 2, :],
                                      ht8_in[:, tb, 2 * j:2 * j + 2, :])

            # ---- ncfw warm-up collective (result unused; no readback —
            # a dependent DMA would block a DMA queue for the ~40us cold
            # collective-setup this exists to absorb) ----
            nc.sync.dma_start(cc2_in[:, :], ones_f[0:8, :])
            nc.gpsimd.collective_compute(
                "AllGather", ALU.bypass,
                replica_groups=[list(range(n_cores))],
                ins=[cc2_in[:, :]], outs=[cc2_out[:, :]])

            # ---- main loop over vocab tiles ----
            accs = [acc_pool.tile([P, TB], F32, name=f"acc{tb}")
                    for tb in range(n_tb)]
            accbs = [acc_pool.tile([P, TB], BF16, name=f"accb{tb}")
                     for tb in range(n_tb)]
            for vt in range(n_vt):
                eT = eT_pre.pop(vt) if vt in eT_pre else alloc_eT(vt)
                pss = [mm_psum.tile([P, TB], F32, tag="mm",
                                    name=f"ps_{vt}_{tb}")
                       for tb in range(n_tb)]
                # tb-outer: each PSUM bank finishes after 4 back-to-back
                # matmuls, so the ScalarE exp drain starts early in the vt
                # and banks recycle before the next vt needs them. (Weight
                # order is free: self-loading matmuls reload every time.)
                for tb in range(n_tb):
                    for j in range(n_jj):
                        lhsT = (eT[:, j, :] if swi
                                else eT[:, 2 * j:2 * j + 2, :])
                        nc.tensor.matmul(
                            pss[tb][:], lhsT=lhsT,
                            rhs=hT[:, tb, 2 * j:2 * j + 2, :],
                            start=(j == 0), stop=(j == n_jj - 1),
                            perf_mode=DRSWI if swi else DR)
                for tb in range(n_tb):
                    if vt == 0:
                        # first tile: write exp straight into the fp32 acc
                        nc.scalar.activation(
                            accs[tb][:], pss[tb][:], AF.Exp,
                            bias=bias_pp[:, vt:vt + 1])
                        continue
                    ex = exp_pool.tile([P, TB], BF16, tag="exp",
                                       name=f"exp_{vt}_{tb}")
                    nc.scalar.activation(
                        ex[:], pss[tb][:], AF.Exp,
                        bias=bias_pp[:, vt:vt + 1])
                    if vt == n_vt - 1:
                        nc.vector.tensor_add(accbs[tb][:], accs[tb][:], ex[:])
                    else:
                        nc.vector.tensor_add(accs[tb][:], accs[tb][:], ex[:])

            # ---- target logits: host pre-gathered rows; dot on device ----
            bg_sb = fin_pool.tile([P, n_gtiles], F32)
            msk_sb = fin_pool.tile([P, n_gtiles], F32)
            nc.sync.dma_start(bg_sb[:], bg_in[:, :])
            nc.sync.dma_start(msk_sb[:], msk_in[:, :])
            dots = fin_pool.tile([P, n_gtiles], F32)
            for g in range(n_gtiles):
                hg = g_pool.tile([P, d_model], F32, tag="grow")
                nc.sync.dma_start(hg[:], hg_in[g])
                eg = g_pool.tile([P, d_model], F32, tag="grow")
                nc.sync.dma_start(eg[:], eg_in[g])
                gsc = g_pool.tile([P, d_model], F32, tag="grow")
                nc.vector.tensor_mul(gsc[:], eg[:], hg[:])
                nc.vector.tensor_reduce(
                    dots[:, g:g + 1], gsc[:],
                    axis=mybir.AxisListType.X, op=ALU.add)
            dsum = fin_pool.tile([P, n_gtiles], F32)
            nc.vector.tensor_add(dsum[:], dots[:], bg_sb[:])
            dmask = fin_pool.tile([P, n_gtiles], F32)
            nc.vector.tensor_mul(dmask[:], dsum[:], msk_sb[:])
            tgt_red = fin_pool.tile([P, 1], F32)
            nc.vector.tensor_reduce(
                tgt_red[:], dmask[:], axis=mybir.AxisListType.X, op=ALU.add)

            # ---- vocab-partition sum: ones-matmuls, rows at part 0/32/64 --
            n_rbank = -(-n_tb // 3)
            red_banks = [mm_psum.tile([P, TB], F32, tag="mm",
                                      name=f"redbank{i}")
                         for i in range(n_rbank)]
            s_sb = fin_pool.tile([P, n_rbank * TB], F32)
            for tb in range(n_tb):
                r = 32 * (tb % 3)
                slot = red_banks[tb // 3][r:r + 1, :]
                nc.tensor.matmul(slot, lhsT=ones_bf[:], rhs=accbs[tb][:],
                                 start=True, stop=True,
                                 skip_group_check=True)
                coff = (tb // 3) * TB
                if tb % 2 == 0:
                    nc.vector.tensor_copy(s_sb[r:r + 1, coff:coff + TB],
                                          slot)
                else:
                    nc.scalar.copy(s_sb[r:r + 1, coff:coff + TB], slot)
                nc.sync.dma_start(cc_in[tb:tb + 1, 0:TB],
                                  s_sb[r:r + 1, coff:coff + TB])
            nc.sync.dma_start(cc_in[:, TB:cc_w], tgt_red[:])

            # ---- allgather S + tgt partials, combine ranks on-chip ----
            agmask = fin_pool.tile([8 * n_tb, n_tb], BF16)
            nc.sync.dma_start(agmask[:], agm_in[:, :])
            nc.gpsimd.collective_compute(
                "AllGather", ALU.bypass,
                replica_groups=[list(range(n_cores))],
                ins=[cc_in[:, :]], outs=[cc_out[:, :]])
            ag_sb = fin_pool.tile([8 * n_tb, cc_w], F32)
            nc.sync.dma_start(ag_sb[:], cc_out[:, :])
            ag_b = fin_pool.tile([8 * n_tb, cc_w], BF16)
            nc.vector.tensor_copy(ag_b[:], ag_sb[:])

            # rank-sum: out[c, n] = sum_r ag[r*n_tb + c, n]
            s_ps = mm_psum.tile([n_tb, TB], F32, tag="mm")
            nc.tensor.matmul(s_ps[:], lhsT=agmask[:], rhs=ag_b[:, 0:TB],
                             start=True, stop=True, skip_group_check=True)
            t_ps = mm_psum.tile([n_tb, tgt_w], F32, tag="mm")
            nc.tensor.matmul(t_ps[:], lhsT=agmask[:], rhs=ag_b[:, TB:cc_w],
                             start=True, stop=True, skip_group_check=True)

            # ---- loss = (sum_t ln(S_t) - sum_t tgt_t) / n_tok ----
            lse = fin_pool.tile([n_tb, TB], F32)
            lse_sum = fin_pool.tile([n_tb, 1], F32)
            nc.scalar.activation(lse[:], s_ps[:], AF.Ln,
                                 accum_out=lse_sum[:])
            tgt_row = fin_pool.tile([n_tb, 1], F32)
            nc.vector.tensor_reduce(tgt_row[:], t_ps[:],
                                    axis=mybir.AxisListType.X, op=ALU.add)
            fvec = fin_pool.tile([n_tb, 1], F32)
            nc.vector.tensor_sub(fvec[:], lse_sum[:], tgt_row[:])
            lp = mm_psum.tile([1, 1], F32, tag="mm")
            nc.tensor.matmul(lp[:], lhsT=ones_f[0:n_tb, :], rhs=fvec[:],
                             start=True, stop=True, skip_group_check=True)
            loss_sb = fin_pool.tile([1, 1], F32)
            nc.scalar.activation(loss_sb[:], lp[:], AF.Copy,
                                 scale=1.0 / float(n_tok))
            nc.sync.dma_start(loss_out[:, :], loss_sb[:])

    nc.finalize()
    return nc


def host_prepare(outputs, word_embeddings, word_biases, labels,
                 n_cores=N_CORES, vs=None, swi=SWI):
    """Shard/pad/transpose/quantize the full inputs into per-core maps."""
    d_model = outputs.shape[-1]
    v_real = word_embeddings.shape[0]
    n_tok = outputs.shape[0] * outputs.shape[1]
    if vs is None:
        vs = -(-v_real // (n_cores * 2 * P)) * 2 * P  # per-core, mult of 256
    v_pad = n_cores * vs
    n_tb = n_tok // TB
    n_dt = d_model // P
    n_vt = vs // P
    n_jj = n_dt // 2

    h = np.ascontiguousarray(
        np.asarray(outputs, dtype=np.float32).reshape(n_tok, d_model))
    e = np.asarray(word_embeddings, dtype=np.float32)
    b = np.asarray(word_biases, dtype=np.float32)
    lab = np.asarray(labels).reshape(-1).astype(np.int64)

    # h^T fp8 [p, tb, d, s] = ALPHA * h[tb*TB+s, d*P+p] (same for all cores)
    th = (h * ALPHA).astype(NP_FP8)
    ht8 = np.ascontiguousarray(
        th.reshape(n_tb, TB, n_dt, P).transpose(3, 0, 2, 1))

    e_pad = np.zeros((v_pad, d_model), dtype=np.float32)
    e_pad[:v_real] = e
    b_pad = np.full(v_pad, BIAS_PAD, dtype=np.float32)
    b_pad[:v_real] = b

    # AllGather rank-combine mask: agmask[r*n_tb + c, c] = 1
    agmask = np.zeros((8 * n_tb, n_tb), dtype=np.float32)
    for c in range(n_tb):
        agmask[c::n_tb, c] = 1.0
    agmask = agmask.astype(ml_dtypes.bfloat16)

    # Per-core gather lists: labels that fall inside each core's shard.
    sels = [np.nonzero((lab >= c * vs) & (lab < (c + 1) * vs))[0]
            for c in range(n_cores)]
    cap = max(max((len(s) for s in sels), default=1), 1)
    n_gtiles = -(-cap // P)
    gcap = n_gtiles * P

    in_maps = []
    for c in range(n_cores):
        es = (e_pad[c * vs:(c + 1) * vs] * BETA).astype(NP_FP8)
        # [vt, p, d, v] = BETA * E[c*vs + vt*P + v, d*P + p]
        et8 = es.reshape(n_vt, P, n_dt, P).transpose(0, 3, 2, 1)
        if swi:
            # [vt, p, j, :]: per DoubleRowSwInterleave, columns reversed,
            # (A, B) d-plane pairs interleaved per column:
            # [A127 B127 A126 B126 ... A0 B0]
            v4 = et8.reshape(n_vt, P, n_jj, 2, P)       # [vt, p, j, ab, v]
            v4 = v4[:, :, :, :, ::-1]                   # reverse columns
            et8 = v4.transpose(0, 1, 2, 4, 3).reshape(n_vt, P, n_jj, 2 * P)
        et8 = np.ascontiguousarray(et8)

        bias_pp = np.ascontiguousarray(
            b_pad[c * vs:(c + 1) * vs].reshape(n_vt, P).T)

        sel = sels[c]
        g_lab = np.zeros(gcap, dtype=np.int64)
        g_tok = np.zeros(gcap, dtype=np.int64)
        g_msk = np.zeros(gcap, dtype=np.float32)
        g_lab[:len(sel)] = lab[sel]
        g_tok[:len(sel)] = sel
        g_msk[:len(sel)] = 1.0
        # tile g, partition p <-> flat index g*P + p
        hg = h[g_tok].reshape(n_gtiles, P, d_model)
        eg = e_pad[g_lab].reshape(n_gtiles, P, d_model)
        bg = np.where(g_msk > 0, b_pad[g_lab], 0.0).astype(np.float32)
        in_maps.append({
            "et8": et8,
            "ht8": ht8,
            "bias_pp": bias_pp,
            "hg": np.ascontiguousarray(hg),
            "eg": np.ascontiguousarray(eg),
            "bg": np.ascontiguousarray(bg.reshape(n_gtiles, P).T),
            "msk": np.ascontiguousarray(g_msk.reshape(n_gtiles, P).T),
            "agmask": agmask,
        })
    meta = dict(n_tok=n_tok, d_model=d_model, vs=vs, n_gtiles=n_gtiles,
                n_cores=n_cores)
    return in_maps, meta


_KERNEL_CACHE = {}


def _get_kernel(meta):
    key = tuple(sorted(meta.items())) + (SWI,)
    if key not in _KERNEL_CACHE:
        _KERNEL_CACHE[key] = build_ce_kernel(**meta, swi=SWI)
    return _KERNEL_CACHE[key]


def kernel(outputs, word_embeddings, word_biases, labels):
    from concourse.bass_utils import run_bass_kernel_spmd

    in_maps, meta = host_prepare(outputs, word_embeddings, word_biases,
                                 labels, n_cores=N_CORES, vs=VS)
    nc = _get_kernel(meta)
    res = run_bass_kernel_spmd(nc, in_maps, list(range(meta["n_cores"])))
    loss = res.results[0]["loss"][0, 0]
    return np.float32(loss)
